# revision 17
# baseline (speedup 1.0000x reference)
"""Trainium2 Bass kernel: 2-layer GRU seq2seq (enc-dec) + 32000-way output projection.

8 NeuronCores, one SPMD program with per-core If branches:
  core 0 (R): all 4 recurrent chains as 2-chain wavefronts (enc L0+L1, then dec
              L0+L1); self-computes layer-1 gate inputs (gi1) from layer-0 outputs.
  core 1 (D): embedding gathers + PE transposes + layer-0 gate inputs (gi0),
              shipped to R via chunked AllGathers; also a projection core.
  cores 1-7:  V-sharded output projection consuming d1 chunks AllGathered from R.

Layout "A" (transposed): per-step state h is SBUF [128, 4*32] bf16 with
[p, kc*32 + b] = h[b, kc*128 + p]; gate tensors (3H) are [128, 12*32] with j-tile
j at cols j*32 (j 0-3 = r, 4-7 = z, 8-11 = n). Weight stationaries are W.T packs:
pack[p, kc*3H + c] = W[c, kc*128 + p]. gi chunk buffers are step-major:
[p, tloc*384 + j*32 + b].
"""

import sys

sys.path.insert(0, "/opt/trn_rl_repo")

import numpy as np
import ml_dtypes

import concourse.bass as bass
import concourse.bacc as bacc
import concourse.mybir as mybir
import concourse.tile as tile
from concourse.tile import add_dep_helper
from concourse import bass_utils

# ---------------------------------------------------------------- constants
S, T, B, V, E, H = 64, 48, 32, 32000, 512, 512
TD = T - 1
P = 128
KC = H // P                     # 4
JT = 3 * H // P                 # 12
G3 = 3 * H                      # 1536
N_CORES = 8
N_PROJ = 7
VC = -(-V // N_PROJ)            # 4572
ENC_CHUNK = 8
ENC_NCHUNK = S // ENC_CHUNK     # 8
DEC_CHUNKS = [12, 12, 12, 8, 3]
GDTOK = 384
GD_NCHUNK = 4
LAG = 2
N_SRC_TILES = (S * B) // P      # 16
N_TRG_TILES = 12
VBANK = 512
NVB = -(-VC // VBANK)           # 9

BF16 = mybir.dt.bfloat16
F32 = mybir.dt.float32
I32 = mybir.dt.int32
AF = mybir.ActivationFunctionType
ALU = mybir.AluOpType

TRACE = False
TRACE_TMPDIR = None
LAST_RESULTS = None
_CACHE = {}


def apv(t, off, dims):
    """Strided free-dim view of a tile AP: dims = [(step, count), ...]."""
    a = t[:]
    return bass.AP(a.tensor, a.offset + off,
                   [list(a.ap[0])] + [[s, c] for s, c in dims])


# ================================================================ device program
def build_program():
    if "nc" in _CACHE:
        return _CACHE["nc"]
    nc = bacc.Bacc("TRN2", target_bir_lowering=False, debug=False,
                   num_devices=N_CORES)

    WSH = [P, KC * G3]
    d_wa = nc.dram_tensor("wa", WSH, BF16, kind="ExternalInput").ap()
    d_wb = nc.dram_tensor("wb", WSH, BF16, kind="ExternalInput").ap()
    d_wc = nc.dram_tensor("wc", WSH, BF16, kind="ExternalInput").ap()
    d_wd = nc.dram_tensor("wd", WSH, BF16, kind="ExternalInput").ap()
    d_we = nc.dram_tensor("we", WSH, BF16, kind="ExternalInput").ap()
    d_wf = nc.dram_tensor("wf", WSH, BF16, kind="ExternalInput").ap()
    d_bn = nc.dram_tensor("bn_all", [1, 4 * KC * P], BF16, kind="ExternalInput").ap()
    d_b1 = nc.dram_tensor("bias1", [1, JT * P], BF16, kind="ExternalInput").ap()
    d_b2 = nc.dram_tensor("bias2", [1, JT * P], BF16, kind="ExternalInput").ap()
    d_emb = nc.dram_tensor("emb", [V, E], BF16, kind="ExternalInput").ap()
    d_emb2 = nc.dram_tensor("emb2", [V, E], BF16, kind="ExternalInput").ap()
    d_idx = nc.dram_tensor("idx", [P, N_SRC_TILES + N_TRG_TILES], I32,
                           kind="ExternalInput").ap()
    d_owt = nc.dram_tensor("outwT", [P, KC * VC], BF16, kind="ExternalInput").ap()
    d_ob = nc.dram_tensor("outb", [1, VC], BF16, kind="ExternalInput").ap()
    d_id = nc.dram_tensor("ident", [P, P], BF16, kind="ExternalInput").ap()
    d_ones = nc.dram_tensor("ones", [1, GDTOK], BF16, kind="ExternalInput").ap()
    out = nc.dram_tensor("out", [TD * B, VC], F32, kind="ExternalOutput").ap()

    RG = [list(range(N_CORES))]

    with tile.TileContext(nc) as tc:
        with (
            tc.tile_pool(name="w", bufs=1) as wp,
            tc.tile_pool(name="sb", bufs=2) as sb,
            tc.tile_pool(name="gib", bufs=1) as gib,
            tc.tile_pool(name="xtp", bufs=3) as xtp,
            tc.tile_pool(name="pst", bufs=2) as pst,
            tc.tile_pool(name="pp1", bufs=1, space="PSUM") as pp1,
            tc.tile_pool(name="pp2", bufs=1, space="PSUM") as pp2,
            tc.tile_pool(name="pp3", bufs=2, space="PSUM") as pp3,
            tc.tile_pool(name="dram", bufs=1, space="DRAM") as dp,
        ):
            cid = nc.partition_id()

            # ---------- DRAM bounce buffers ----------
            ge_in = [dp.tile([P, JT * ENC_CHUNK * B], BF16, tag=f"gei{c}", name=f"gei{c}")
                     for c in range(ENC_NCHUNK)]
            ge_out = [dp.tile([N_CORES * P, JT * ENC_CHUNK * B], BF16, tag=f"geo{c}", name=f"geo{c}")
                      for c in range(ENC_NCHUNK)]
            gd_in = [dp.tile([P, JT * GDTOK], BF16, tag=f"gdi{c}", name=f"gdi{c}")
                     for c in range(GD_NCHUNK)]
            gd_out = [dp.tile([N_CORES * P, JT * GDTOK], BF16, tag=f"gdo{c}", name=f"gdo{c}")
                      for c in range(GD_NCHUNK)]
            d1_in = [dp.tile([P, 12 * P], BF16, tag=f"d1i{c}", name=f"d1i{c}")
                     for c in range(len(DEC_CHUNKS))]
            d1_out = [dp.tile([N_CORES * P, 12 * P], BF16, tag=f"d1o{c}", name=f"d1o{c}")
                      for c in range(len(DEC_CHUNKS))]

            # ---------- persistent SBUF tiles (same addresses on every core) ----
            w_a = wp.tile(WSH, BF16, tag="wa")
            w_b = wp.tile(WSH, BF16, tag="wb")
            w_e = wp.tile(WSH, BF16, tag="we")
            bn_all = wp.tile([1, 4 * KC * P], BF16, tag="bn")
            bias1 = wp.tile([1, JT * P], BF16, tag="b1")
            bias2 = wp.tile([1, JT * P], BF16, tag="b2")
            ident = wp.tile([P, P], BF16, tag="ident")
            idx_t = wp.tile([P, N_SRC_TILES + N_TRG_TILES], I32, tag="idx")
            outwT = wp.tile([P, KC * VC], BF16, tag="owt")
            outb = wp.tile([1, VC], BF16, tag="ob")
            ones = wp.tile([1, GDTOK], BF16, tag="ones")
            y0ring = wp.tile([P, 2 * P], BF16, tag="y0r")
            y1ring = wp.tile([P, 2 * P], BF16, tag="y1r")
            gi1ring = wp.tile([P, 4 * JT * B], BF16, tag="g1r")
            anch = wp.tile([1, 16], BF16, tag="anch")

            nc.sync.dma_start(idx_t[:], d_idx[:])
            nc.sync.dma_start(ident[:], d_id[:])
            nc.sync.dma_start(bias1[:], d_b1[:])
            nc.sync.dma_start(bias2[:], d_b2[:])
            nc.sync.dma_start(bn_all[:], d_bn[:])
            nc.sync.dma_start(ones[:], d_ones[:])
            nc.sync.dma_start(w_a[:], d_wa[:])
            nc.sync.dma_start(w_b[:], d_wb[:])
            nc.sync.dma_start(w_e[:], d_we[:])
            with tc.If(cid == 0) as cldw:
                pass
            with cldw.Else():
                nc.sync.dma_start(outb[:], d_ob[:])
                nc.sync.dma_start(outwT[:], d_owt[:])
            nc.gpsimd.memset(y0ring[:], 0.0)
            nc.gpsimd.memset(y1ring[:], 0.0)

            # ========== D: gather + transpose one 128-token tile ==========
            def d_gather_xT(emb_ap, m):
                g = xtp.tile([P, E], BF16, tag="gath", bufs=2)
                nc.gpsimd.indirect_dma_start(
                    out=g[:], out_offset=None, in_=emb_ap[:],
                    in_offset=bass.IndirectOffsetOnAxis(ap=idx_t[:, m:m + 1], axis=0),
                )
                ps = pp2.tile([P, E], BF16, space="PSUM", tag="pbig")
                for kc in range(KC):
                    nc.tensor.transpose(out=ps[:, kc * P:(kc + 1) * P],
                                        in_=g[:, kc * P:(kc + 1) * P],
                                        identity=ident[:])
                xt = xtp.tile([P, E], BF16, tag="xT", bufs=7)
                nc.scalar.activation(xt[:], ps[:], AF.Copy)
                return xt

            # ========== D: gi0 for a chunk of token tiles (step-major dst) ==========
            def d_gi0_chunk(xts, w_ih, bias_rows, dst):
                ntiles = len(xts)
                ntok = ntiles * P
                nsteps = ntok // B
                for j in range(JT):
                    ps = pp1.tile([P, GDTOK], F32, space="PSUM",
                                  tag="prz0" if j % 2 == 0 else "prz1")
                    psv = ps[:, 0:ntok]
                    nc.tensor.matmul(psv, lhsT=bias_rows[0:1, j * P:(j + 1) * P],
                                     rhs=ones[:, 0:ntok], start=True, stop=False)
                    for kc in range(KC):
                        for mm in range(ntiles):
                            nc.tensor.matmul(
                                ps[:, mm * P:(mm + 1) * P],
                                lhsT=w_ih[:, kc * G3 + j * P:kc * G3 + (j + 1) * P],
                                rhs=xts[mm][:, kc * P:(kc + 1) * P],
                                start=False,
                                stop=(kc == KC - 1 and mm == ntiles - 1),
                            )
                    iv = apv(ps, 0, [(B, nsteps), (1, B)])
                    ov = apv(dst, j * B, [(JT * B, nsteps), (1, B)])
                    nc.scalar.activation(ov, iv, AF.Copy)

            # ========== R: one GRU layer-step (layout A) ==========
            def r_step(w_hh, bn_row0, ring, prev_slot, gi_ap, tagp):
                cur = 1 - prev_slot
                h_prev = ring[:, prev_slot * P:(prev_slot + 1) * P]
                prz = pp1.tile([P, 8 * B], F32, space="PSUM", tag=f"prz{tagp}")
                pn = pp1.tile([P, 4 * B], F32, space="PSUM", tag=f"pn{tagp}")
                for j in range(8):
                    for kc in range(KC):
                        nc.tensor.matmul(
                            prz[:, j * B:(j + 1) * B],
                            lhsT=w_hh[:, kc * G3 + j * P:kc * G3 + (j + 1) * P],
                            rhs=h_prev[:, kc * B:(kc + 1) * B],
                            start=(kc == 0), stop=(kc == KC - 1),
                        )
                for jj in range(4):
                    nc.tensor.matmul(pn[:, jj * B:(jj + 1) * B],
                                     lhsT=bn_all[0:1, (bn_row0 + jj) * P:(bn_row0 + jj + 1) * P],
                                     rhs=ones[:, 0:B], start=True, stop=False)
                    j = 8 + jj
                    for kc in range(KC):
                        nc.tensor.matmul(
                            pn[:, jj * B:(jj + 1) * B],
                            lhsT=w_hh[:, kc * G3 + j * P:kc * G3 + (j + 1) * P],
                            rhs=h_prev[:, kc * B:(kc + 1) * B],
                            start=False, stop=(kc == KC - 1),
                        )
                rz_pre = sb.tile([P, 8 * B], BF16, tag=f"rzp{tagp}")
                nc.vector.tensor_add(rz_pre[:], prz[:], gi_ap[:, 0:8 * B])
                rz = sb.tile([P, 8 * B], BF16, tag=f"rz{tagp}")
                nc.scalar.activation(rz[:], rz_pre[:], AF.Sigmoid)
                rhn = sb.tile([P, 4 * B], BF16, tag=f"rhn{tagp}")
                nc.vector.tensor_mul(rhn[:], rz[:, 0:4 * B], pn[:])
                npre = sb.tile([P, 4 * B], BF16, tag=f"npre{tagp}")
                nc.vector.tensor_add(npre[:], rhn[:], gi_ap[:, 8 * B:12 * B])
                cand = sb.tile([P, 4 * B], BF16, tag=f"cand{tagp}")
                nc.scalar.activation(cand[:], npre[:], AF.Tanh)
                zh = sb.tile([P, 4 * B], BF16, tag=f"zh{tagp}")
                nc.gpsimd.tensor_mul(zh[:], rz[:, 4 * B:8 * B], h_prev[:])
                zc = sb.tile([P, 4 * B], BF16, tag=f"zc{tagp}")
                nc.vector.tensor_mul(zc[:], rz[:, 4 * B:8 * B], cand[:])
                t2 = sb.tile([P, 4 * B], BF16, tag=f"t2{tagp}")
                nc.vector.tensor_sub(t2[:], cand[:], zc[:])
                nc.vector.tensor_add(ring[:, cur * P:(cur + 1) * P], t2[:], zh[:])
                return cur

            # ========== R: gi1 for 1-2 L0 output steps ==========
            def r_gi1(w_ih, bias_rows, src_ring, slots, dst_slot0):
                nsteps = len(slots)
                ntok = nsteps * B
                pg = pp2.tile([P, JT * 2 * B], F32, space="PSUM", tag="pbig")
                for j in range(JT):
                    jb = j * 2 * B
                    nc.tensor.matmul(pg[:, jb:jb + ntok],
                                     lhsT=bias_rows[0:1, j * P:(j + 1) * P],
                                     rhs=ones[:, 0:ntok], start=True, stop=False)
                    for kc in range(KC):
                        if nsteps == 2:
                            nc.tensor.matmul(
                                pg[:, jb:jb + 2 * B],
                                lhsT=w_ih[:, kc * G3 + j * P:kc * G3 + (j + 1) * P],
                                rhs=apv(src_ring, kc * B, [(P, 2), (1, B)]),
                                start=False, stop=(kc == KC - 1),
                            )
                        else:
                            nc.tensor.matmul(
                                pg[:, jb:jb + B],
                                lhsT=w_ih[:, kc * G3 + j * P:kc * G3 + (j + 1) * P],
                                rhs=src_ring[:, slots[0] * P + kc * B:
                                             slots[0] * P + kc * B + B],
                                start=False, stop=(kc == KC - 1),
                            )
                for i in range(nsteps):
                    iv = apv(pg, i * B, [(2 * B, JT), (1, B)])
                    ov = apv(gi1ring, (dst_slot0 + i) * JT * B, [(B, JT), (1, B)])
                    if i == 0:
                        nc.vector.tensor_copy(ov, iv)
                    else:
                        nc.scalar.activation(ov, iv, AF.Copy)

            # ========== proj: one d1 AG chunk on cores 1..7 ==========
            def proj_chunk(k, tok0, ntok):
                dsb = pst.tile([P, 12 * P], BF16, tag="d1sb", bufs=1)
                nc.sync.dma_start(dsb[:], d1_out[k][0:P, :])
                nmt = -(-ntok // P)
                for m2 in range(nmt):
                    mrows = min(P, ntok - m2 * P)
                    msteps = mrows // B
                    for vb in range(NVB):
                        vw = min(VBANK, VC - vb * VBANK)
                        ps = pp3.tile([P, VBANK], F32, space="PSUM", tag="pproj")
                        psv = ps[0:mrows, 0:vw]
                        nc.tensor.matmul(psv, lhsT=ones[:, 0:mrows],
                                         rhs=outb[:, vb * VBANK:vb * VBANK + vw],
                                         start=True, stop=False)
                        for kc in range(KC):
                            lhs = dsb[:, kc * 12 * B + m2 * P:
                                      kc * 12 * B + m2 * P + mrows]
                            nc.tensor.matmul(
                                psv, lhsT=lhs,
                                rhs=outwT[:, kc * VC + vb * VBANK:
                                          kc * VC + vb * VBANK + vw],
                                start=False, stop=(kc == KC - 1),
                            )
                        o = pst.tile([P, VBANK], F32, tag="postage")
                        nc.vector.tensor_copy(o[0:mrows, 0:vw], psv)
                        nc.sync.dma_start(
                            out[tok0 + m2 * P:tok0 + m2 * P + mrows,
                                vb * VBANK:vb * VBANK + vw],
                            o[0:mrows, 0:vw])

            # =========================================================
            # emission
            # =========================================================
            rs = {"s0": 1, "s1": 1}
            gd_sb = [None] * GD_NCHUNK

            def emit_r_steps(t_lo, t_hi, gi0_of, whh0, whh1, wih1, biasL1,
                             bn0_row0, bn1_row0, nsteps_total, d1_sink=None):
                for t in range(t_lo, t_hi):
                    if t < nsteps_total:
                        rs["s0"] = r_step(whh0, bn0_row0, y0ring, rs["s0"],
                                          gi0_of(t), "0")
                        if t % 2 == 1 or t == nsteps_total - 1:
                            g = t // 2
                            slots = ([(t - 1) % 2, t % 2] if t % 2 == 1
                                     else [t % 2])
                            r_gi1(wih1, biasL1, y0ring, slots, (g % 2) * 2)
                    tl1 = t - LAG
                    if 0 <= tl1 < nsteps_total:
                        g = tl1 // 2
                        off = ((g % 2) * 2 + tl1 % 2) * JT * B
                        rs["s1"] = r_step(whh1, bn1_row0, y1ring, rs["s1"],
                                          gi1ring[:, off:off + JT * B], "1")
                        if d1_sink is not None:
                            ck, tloc = d1_sink(tl1)
                            nc.sync.dma_start(
                                apv(d1_in[ck], tloc * B, [(12 * B, KC), (1, B)]),
                                apv(y1ring, rs["s1"] * P, [(B, KC), (1, B)]))

            # ---------------- encoder phase ----------------
            xt_store = {}
            enc_anchors = []

            def d_needs(c):
                tiles = [2 * c, 2 * c + 1]
                if 3 <= c <= 6:
                    kk = c - 3
                    tiles += [N_SRC_TILES + 3 * kk + i for i in range(3)]
                return tiles

            def d_gather_tiles(tiles):
                for m in tiles:
                    emb_ap = d_emb if m < N_SRC_TILES else d_emb2
                    xt_store[m] = d_gather_xT(emb_ap, m)

            cc_pending = []

            def emit_d_chunk(c):
                with tc.If(cid == 1):
                    if c == 0:
                        d_gather_tiles([0, 1])
                    g = gib.tile([P, JT * GDTOK], BF16, tag="gD", name="gD")
                    d_gi0_chunk([xt_store[2 * c], xt_store[2 * c + 1]],
                                w_a, bias1, g)
                    if c + 1 < ENC_NCHUNK:
                        d_gather_tiles([2 * (c + 1), 2 * (c + 1) + 1])
                    nc.sync.dma_start(ge_in[c][:], g[:, 0:JT * ENC_CHUNK * B])
                    if 3 <= c <= 6:
                        kk = c - 3
                        d_gather_tiles([N_SRC_TILES + 3 * kk + i for i in range(3)])
                        xts2 = [xt_store[N_SRC_TILES + 3 * kk + i] for i in range(3)]
                        g2 = gib.tile([P, JT * GDTOK], BF16, tag="gD", name="gD2")
                        d_gi0_chunk(xts2, w_b, bias2, g2)
                        nc.sync.dma_start(gd_in[kk][:], g2[:])
                cc = nc.gpsimd.collective_compute(
                    "AllGather", ALU.bypass, replica_groups=RG,
                    ins=[ge_in[c].opt()], outs=[ge_out[c].opt()])
                cc_pending.append(cc)
                if c >= 4:
                    cc2 = nc.gpsimd.collective_compute(
                        "AllGather", ALU.bypass, replica_groups=RG,
                        ins=[gd_in[c - 4].opt()], outs=[gd_out[c - 4].opt()])
                    cc_pending.append(cc2)

            for c2 in range(ENC_NCHUNK // 2):
                emit_d_chunk(2 * c2)
                emit_d_chunk(2 * c2 + 1)
                with tc.If(cid == 0):
                    a = nc.gpsimd.memset(anch[0:1, 0:8], 0.0)
                    enc_anchors.append(a)
                    for ccp in cc_pending:
                        add_dep_helper(a.ins, ccp.ins, sync=False)
                    cc_pending.clear()
                    gsbs = []
                    for c in (2 * c2, 2 * c2 + 1):
                        gsb = gib.tile([P, JT * GDTOK], BF16, tag=f"ge{c % 2}",
                                       name=f"gsb{c}")
                        nc.sync.dma_start(gsb[:, 0:JT * ENC_CHUNK * B],
                                          ge_out[c][P:2 * P, :])
                        gsbs.append(gsb)

                    def gi0_of(t, _g=gsbs, _c2=c2):
                        tloc = t - _c2 * 2 * ENC_CHUNK
                        return _g[tloc // ENC_CHUNK][
                            :, (tloc % ENC_CHUNK) * JT * B:
                            (tloc % ENC_CHUNK + 1) * JT * B]

                    emit_r_steps(c2 * 2 * ENC_CHUNK, (c2 + 1) * 2 * ENC_CHUNK,
                                 gi0_of, w_a, w_b, w_e, bias1, 0, KC, S)

            # ---------------- decoder phase ----------------
            dec_starts = [0]
            for L in DEC_CHUNKS:
                dec_starts.append(dec_starts[-1] + L)

            def d1_sink(t1):
                for ck in range(len(DEC_CHUNKS)):
                    if t1 < dec_starts[ck + 1]:
                        return ck, t1 - dec_starts[ck]
                raise AssertionError

            nK = len(DEC_CHUNKS)
            w_c2 = w_d2 = w_f2 = None
            d1_cc = [None] * nK
            for k in range(nK):
                t_lo, t_hi = dec_starts[k], dec_starts[k + 1]
                with tc.If(cid == 0):
                    if k == 0:
                        # encoder L1 tail
                        emit_r_steps(S, S + LAG, None,
                                     w_a, w_b, w_e, bias1, 0, KC, S)
                        # load decoder weights (own tags, loaded late)
                        w_c2 = wp.tile(WSH, BF16, tag="wc", name="wc2")
                        nc.sync.dma_start(w_c2[:], d_wc[:])
                        w_d2 = wp.tile(WSH, BF16, tag="wd", name="wd2")
                        nc.sync.dma_start(w_d2[:], d_wd[:])
                        w_f2 = wp.tile(WSH, BF16, tag="wf", name="wf2")
                        nc.sync.dma_start(w_f2[:], d_wf[:])
                    a = nc.gpsimd.memset(anch[0:1, 0:8], 0.0)
                    if k >= 2 and d1_cc[k - 2] is not None:
                        add_dep_helper(a.ins, d1_cc[k - 2].ins, sync=False)
                    loads = {0: [0, 1], 1: [2], 2: [3]}.get(k, [])
                    for kk in loads:
                        gg = gib.tile([P, JT * GDTOK], BF16, tag=f"ge{kk % 2}",
                                      name=f"gdl{kk}")
                        nc.sync.dma_start(gg[:], gd_out[kk][P:2 * P, :])
                        gd_sb[kk % 2] = (kk, gg)

                    def gi0d_of(t):
                        kk = t // 12
                        tloc = t - kk * 12
                        kk2, gg = gd_sb[kk % 2]
                        assert kk2 == kk
                        return gg[:, tloc * JT * B:(tloc + 1) * JT * B]

                    emit_r_steps(t_lo, t_hi, gi0d_of,
                                 w_c2, w_d2, w_f2, bias2, 2 * KC, 3 * KC, TD,
                                 d1_sink=d1_sink)
                    if k == nK - 1:
                        emit_r_steps(TD, TD + LAG, None,
                                     w_c2, w_d2, w_f2, bias2, 2 * KC, 3 * KC, TD,
                                     d1_sink=d1_sink)
                if k >= 1:
                    kk = k - 1
                    d1_cc[kk] = nc.gpsimd.collective_compute(
                        "AllGather", ALU.bypass, replica_groups=RG,
                        ins=[d1_in[kk].opt()], outs=[d1_out[kk].opt()])
                    with tc.If(cid == 0) as cp:
                        pass
                    with cp.Else():
                        proj_chunk(kk, dec_starts[kk] * B,
                                   DEC_CHUNKS[kk] * B)
            # final d1 chunk
            kk = nK - 1
            nc.gpsimd.collective_compute(
                "AllGather", ALU.bypass, replica_groups=RG,
                ins=[d1_in[kk].opt()], outs=[d1_out[kk].opt()])
            with tc.If(cid == 0) as cp:
                pass
            with cp.Else():
                proj_chunk(kk, dec_starts[kk] * B, DEC_CHUNKS[kk] * B)

    nc.compile()
    _CACHE["nc"] = nc
    return nc


# ================================================================ host side
def _bf16(x):
    return np.asarray(x, dtype=np.float32).astype(ml_dtypes.bfloat16)


def pack_wT(W):
    """W [out_dim, in_dim] -> [128, (in_dim/128)*out_dim] bf16 stationary pack."""
    W = np.asarray(W, np.float32)
    out_dim, in_dim = W.shape
    kc = in_dim // P
    Wt = W.T.reshape(kc, P, out_dim)
    return _bf16(np.concatenate([Wt[i] for i in range(kc)], axis=1))


def pack_bias_rows(bih, bhh):
    """gi-side bias rows [12, 128]: rz tiles bih+bhh, n tiles bih only."""
    bih = np.asarray(bih, np.float32)
    bhh = np.asarray(bhh, np.float32)
    rows = np.zeros((JT, P), np.float32)
    for j in range(JT):
        seg = bih[j * P:(j + 1) * P].copy()
        if j < 8:
            seg = seg + bhh[j * P:(j + 1) * P]
        rows[j] = seg
    return _bf16(rows.reshape(1, JT * P))


def pack_bn_rows(bhh):
    return _bf16(np.asarray(bhh, np.float32)[2 * H:3 * H].reshape(1, KC * P))


def pack_idx(src, trg):
    src_f = np.asarray(src, np.int64).reshape(-1)
    trg_f = np.asarray(trg, np.int64)[:TD].reshape(-1)
    trg_f = np.concatenate([trg_f, np.zeros(N_TRG_TILES * P - TD * B, np.int64)])
    cols = [src_f[m * P:(m + 1) * P] for m in range(N_SRC_TILES)]
    cols += [trg_f[m * P:(m + 1) * P] for m in range(N_TRG_TILES)]
    return np.stack(cols, axis=1).astype(np.int32)


def pack_outwT(out_W, core):
    vbase = (core - 1) * VC
    sl = np.zeros((VC, H), np.float32)
    real = np.asarray(out_W, np.float32)[vbase:min(vbase + VC, V)]
    sl[:real.shape[0]] = real
    slT = sl.T.reshape(KC, P, VC)
    return _bf16(np.concatenate([slT[i] for i in range(KC)], axis=1))


def pack_outb(out_b, core):
    vbase = (core - 1) * VC
    sl = np.zeros((VC,), np.float32)
    real = np.asarray(out_b, np.float32)[vbase:min(vbase + VC, V)]
    sl[:real.shape[0]] = real
    return _bf16(sl[None, :])


def make_in_maps(inputs):
    z_w = np.zeros((P, KC * G3), ml_dtypes.bfloat16)
    z_emb = np.zeros((V, E), ml_dtypes.bfloat16)
    z_owt = np.zeros((P, KC * VC), ml_dtypes.bfloat16)
    z_ob = np.zeros((1, VC), ml_dtypes.bfloat16)
    z_bn = np.zeros((1, 4 * KC * P), ml_dtypes.bfloat16)
    z_b = np.zeros((1, JT * P), ml_dtypes.bfloat16)
    z_idx = np.zeros((P, N_SRC_TILES + N_TRG_TILES), np.int32)
    ident = _bf16(np.eye(P))
    ones_arr = np.ones((1, GDTOK), ml_dtypes.bfloat16)

    emb_e = _bf16(inputs["enc_emb"])
    emb_d = _bf16(inputs["dec_emb"])
    idx = pack_idx(inputs["src"], inputs["trg"])

    in_maps = []
    for core in range(N_CORES):
        m = {
            "wa": z_w, "wb": z_w, "wc": z_w, "wd": z_w, "we": z_w, "wf": z_w,
            "bn_all": z_bn, "bias1": z_b, "bias2": z_b,
            "emb": z_emb, "emb2": z_emb, "idx": z_idx,
            "outwT": z_owt, "outb": z_ob,
            "ident": ident, "ones": ones_arr,
        }
        if core == 0:
            m["wa"] = pack_wT(inputs["enc_Whh0"])
            m["wb"] = pack_wT(inputs["enc_Whh1"])
            m["wc"] = pack_wT(inputs["dec_Whh0"])
            m["wd"] = pack_wT(inputs["dec_Whh1"])
            m["we"] = pack_wT(inputs["enc_Wih1"])
            m["wf"] = pack_wT(inputs["dec_Wih1"])
            m["bn_all"] = np.concatenate([
                pack_bn_rows(inputs["enc_bhh0"]),
                pack_bn_rows(inputs["enc_bhh1"]),
                pack_bn_rows(inputs["dec_bhh0"]),
                pack_bn_rows(inputs["dec_bhh1"]),
            ], axis=1)
            m["bias1"] = pack_bias_rows(inputs["enc_bih1"], inputs["enc_bhh1"])
            m["bias2"] = pack_bias_rows(inputs["dec_bih1"], inputs["dec_bhh1"])
        if core == 1:
            m["wa"] = pack_wT(inputs["enc_Wih0"])
            m["wb"] = pack_wT(inputs["dec_Wih0"])
            m["bias1"] = pack_bias_rows(inputs["enc_bih0"], inputs["enc_bhh0"])
            m["bias2"] = pack_bias_rows(inputs["dec_bih0"], inputs["dec_bhh0"])
            m["emb"] = emb_e
            m["emb2"] = emb_d
            m["idx"] = idx
        if core >= 1:
            m["outwT"] = pack_outwT(inputs["out_W"], core)
            m["outb"] = pack_outb(inputs["out_b"], core)
        in_maps.append(m)
    return in_maps


def kernel(**inputs):
    global LAST_RESULTS
    nc = build_program()
    in_maps = make_in_maps(inputs)
    kw = {}
    if TRACE:
        try:
            import ntff_shim
            ntff_shim.install()
        except Exception:
            pass
        kw["trace"] = True
        if TRACE_TMPDIR:
            kw["tmpdir"] = TRACE_TMPDIR
    res = bass_utils.run_bass_kernel_spmd(nc, in_maps,
                                          core_ids=list(range(N_CORES)), **kw)
    LAST_RESULTS = res
    parts = [np.asarray(res.results[c]["out"]) for c in range(1, N_CORES)]
    logits = np.concatenate(parts, axis=1)[:, :V]
    return logits.reshape(TD, B, V).astype(np.float32)


# revision 18
# speedup vs baseline: 1.1533x; 1.1533x over previous
"""Trainium2 Bass kernel: 2-layer GRU seq2seq (enc-dec) + 32000-way output projection.

8 NeuronCores, one SPMD program with per-core If branches:
  core 0 (R): all 4 recurrent chains as 2-chain wavefronts (enc L0+L1, then dec
              L0+L1); self-computes layer-1 gate inputs (gi1) from layer-0 outputs.
  core 1 (D): embedding gathers + PE transposes + layer-0 gate inputs (gi0),
              shipped to R via chunked AllGathers; also a projection core.
  cores 1-7:  V-sharded output projection consuming d1 chunks AllGathered from R.

Layout "A" (transposed): per-step state h is SBUF [128, 4*32] bf16 with
[p, kc*32 + b] = h[b, kc*128 + p]; gate tensors (3H) are [128, 12*32] with j-tile
j at cols j*32 (j 0-3 = r, 4-7 = z, 8-11 = n). Weight stationaries are W.T packs:
pack[p, kc*3H + c] = W[c, kc*128 + p]. gi chunk buffers are step-major:
[p, tloc*384 + j*32 + b].
"""

import sys

sys.path.insert(0, "/opt/trn_rl_repo")

import numpy as np
import ml_dtypes

import concourse.bass as bass
import concourse.bacc as bacc
import concourse.mybir as mybir
import concourse.tile as tile
from concourse.tile import add_dep_helper
from concourse import bass_utils

# ---------------------------------------------------------------- constants
S, T, B, V, E, H = 64, 48, 32, 32000, 512, 512
TD = T - 1
P = 128
KC = H // P                     # 4
JT = 3 * H // P                 # 12
G3 = 3 * H                      # 1536
N_CORES = 8
N_PROJ = 7
VC = -(-V // N_PROJ)            # 4572
ENC_CHUNK = 8
ENC_NCHUNK = S // ENC_CHUNK     # 8
DEC_CHUNKS = [12, 12, 12, 8, 3]
GDTOK = 384
GD_NCHUNK = 4
LAG = 2
N_SRC_TILES = (S * B) // P      # 16
N_TRG_TILES = 12
VBANK = 512
NVB = -(-VC // VBANK)           # 9

BF16 = mybir.dt.bfloat16
F32 = mybir.dt.float32
I32 = mybir.dt.int32
AF = mybir.ActivationFunctionType
ALU = mybir.AluOpType

TRACE = False
TRACE_TMPDIR = None
LAST_RESULTS = None
_CACHE = {}


def apv(t, off, dims):
    """Strided free-dim view of a tile AP: dims = [(step, count), ...]."""
    a = t[:]
    return bass.AP(a.tensor, a.offset + off,
                   [list(a.ap[0])] + [[s, c] for s, c in dims])


# ================================================================ device program
def build_program():
    if "nc" in _CACHE:
        return _CACHE["nc"]
    nc = bacc.Bacc("TRN2", target_bir_lowering=False, debug=False,
                   num_devices=N_CORES)

    WSH = [P, KC * G3]
    d_wa = nc.dram_tensor("wa", WSH, BF16, kind="ExternalInput").ap()
    d_wb = nc.dram_tensor("wb", WSH, BF16, kind="ExternalInput").ap()
    d_wc = nc.dram_tensor("wc", WSH, BF16, kind="ExternalInput").ap()
    d_wd = nc.dram_tensor("wd", WSH, BF16, kind="ExternalInput").ap()
    d_we = nc.dram_tensor("we", WSH, BF16, kind="ExternalInput").ap()
    d_wf = nc.dram_tensor("wf", WSH, BF16, kind="ExternalInput").ap()
    d_bn = nc.dram_tensor("bn_all", [1, 4 * KC * P], BF16, kind="ExternalInput").ap()
    d_b1 = nc.dram_tensor("bias1", [1, JT * P], BF16, kind="ExternalInput").ap()
    d_b2 = nc.dram_tensor("bias2", [1, JT * P], BF16, kind="ExternalInput").ap()
    d_emb = nc.dram_tensor("emb", [V, E], BF16, kind="ExternalInput").ap()
    d_emb2 = nc.dram_tensor("emb2", [V, E], BF16, kind="ExternalInput").ap()
    d_idx = nc.dram_tensor("idx", [P, N_SRC_TILES + N_TRG_TILES], I32,
                           kind="ExternalInput").ap()
    d_owt = nc.dram_tensor("outwT", [P, KC * VC], BF16, kind="ExternalInput").ap()
    d_ob = nc.dram_tensor("outb", [1, VC], BF16, kind="ExternalInput").ap()
    d_id = nc.dram_tensor("ident", [P, P], BF16, kind="ExternalInput").ap()
    d_ones = nc.dram_tensor("ones", [1, GDTOK], BF16, kind="ExternalInput").ap()
    out = nc.dram_tensor("out", [TD * B, VC], F32, kind="ExternalOutput").ap()

    RG = [list(range(N_CORES))]
    RG2 = [[0, 1], [2, 3], [4, 5], [6, 7]]

    with tile.TileContext(nc) as tc:
        with (
            tc.tile_pool(name="w", bufs=1) as wp,
            tc.tile_pool(name="sb", bufs=2) as sb,
            tc.tile_pool(name="gib", bufs=1) as gib,
            tc.tile_pool(name="xtp", bufs=3) as xtp,
            tc.tile_pool(name="pst", bufs=2) as pst,
            tc.tile_pool(name="pp1", bufs=1, space="PSUM") as pp1,
            tc.tile_pool(name="pp2", bufs=1, space="PSUM") as pp2,
            tc.tile_pool(name="pp3", bufs=2, space="PSUM") as pp3,
            tc.tile_pool(name="dram", bufs=1, space="DRAM") as dp,
        ):
            cid = nc.partition_id()

            # ---------- DRAM bounce buffers ----------
            ge_in = [dp.tile([P, JT * ENC_CHUNK * B], BF16, tag=f"gei{c}", name=f"gei{c}")
                     for c in range(ENC_NCHUNK)]
            ge_out = [dp.tile([2 * P, JT * ENC_CHUNK * B], BF16, tag=f"geo{c}", name=f"geo{c}")
                      for c in range(ENC_NCHUNK)]
            gd_in = [dp.tile([P, JT * GDTOK], BF16, tag=f"gdi{c}", name=f"gdi{c}")
                     for c in range(GD_NCHUNK)]
            gd_out = [dp.tile([2 * P, JT * GDTOK], BF16, tag=f"gdo{c}", name=f"gdo{c}")
                      for c in range(GD_NCHUNK)]
            d1_in = [dp.tile([P, 12 * P], BF16, tag=f"d1i{c}", name=f"d1i{c}")
                     for c in range(len(DEC_CHUNKS))]
            d1_out = [dp.tile([N_CORES * P, 12 * P], BF16, tag=f"d1o{c}", name=f"d1o{c}")
                      for c in range(len(DEC_CHUNKS))]

            # ---------- persistent SBUF tiles (same addresses on every core) ----
            w_a = wp.tile(WSH, BF16, tag="wa")
            w_b = wp.tile(WSH, BF16, tag="wb")
            w_e = wp.tile(WSH, BF16, tag="we")
            bn_all = wp.tile([1, 4 * KC * P], BF16, tag="bn")
            bias1 = wp.tile([1, JT * P], BF16, tag="b1")
            bias2 = wp.tile([1, JT * P], BF16, tag="b2")
            ident = wp.tile([P, P], BF16, tag="ident")
            idx_t = wp.tile([P, N_SRC_TILES + N_TRG_TILES], I32, tag="idx")
            outwT = wp.tile([P, KC * VC], BF16, tag="owt")
            outb = wp.tile([1, VC], BF16, tag="ob")
            ones = wp.tile([1, GDTOK], BF16, tag="ones")
            y0ring = wp.tile([P, 2 * P], BF16, tag="y0r")
            y1ring = wp.tile([P, 2 * P], BF16, tag="y1r")
            gi1ring = wp.tile([P, 4 * JT * B], BF16, tag="g1r")
            anch = wp.tile([1, 16], BF16, tag="anch")

            nc.sync.dma_start(idx_t[:], d_idx[:])
            nc.sync.dma_start(ident[:], d_id[:])
            nc.sync.dma_start(bias1[:], d_b1[:])
            nc.sync.dma_start(bias2[:], d_b2[:])
            nc.sync.dma_start(bn_all[:], d_bn[:])
            nc.sync.dma_start(ones[:], d_ones[:])
            nc.sync.dma_start(w_a[:], d_wa[:])
            nc.sync.dma_start(w_b[:], d_wb[:])
            nc.sync.dma_start(w_e[:], d_we[:])
            with tc.If(cid == 0) as cldw:
                pass
            with cldw.Else():
                nc.sync.dma_start(outb[:], d_ob[:])
                nc.sync.dma_start(outwT[:], d_owt[:])
            nc.gpsimd.memset(y0ring[:], 0.0)
            nc.gpsimd.memset(y1ring[:], 0.0)

            # ========== D: gather + transpose one 128-token tile ==========
            def d_gather_xT(emb_ap, m):
                g = xtp.tile([P, E], BF16, tag="gath", bufs=2)
                nc.gpsimd.indirect_dma_start(
                    out=g[:], out_offset=None, in_=emb_ap[:],
                    in_offset=bass.IndirectOffsetOnAxis(ap=idx_t[:, m:m + 1], axis=0),
                )
                ps = pp2.tile([P, E], BF16, space="PSUM", tag="pbig")
                for kc in range(KC):
                    nc.tensor.transpose(out=ps[:, kc * P:(kc + 1) * P],
                                        in_=g[:, kc * P:(kc + 1) * P],
                                        identity=ident[:])
                xt = xtp.tile([P, E], BF16, tag="xT", bufs=7)
                nc.scalar.activation(xt[:], ps[:], AF.Copy)
                return xt

            # ========== D: gi0 for a chunk of token tiles (step-major dst) ==========
            def d_gi0_chunk(xts, w_ih, bias_rows, dst):
                ntiles = len(xts)
                ntok = ntiles * P
                nsteps = ntok // B
                for j in range(JT):
                    ps = pp1.tile([P, GDTOK], F32, space="PSUM",
                                  tag="prz0" if j % 2 == 0 else "prz1")
                    psv = ps[:, 0:ntok]
                    nc.tensor.matmul(psv, lhsT=bias_rows[0:1, j * P:(j + 1) * P],
                                     rhs=ones[:, 0:ntok], start=True, stop=False)
                    for kc in range(KC):
                        for mm in range(ntiles):
                            nc.tensor.matmul(
                                ps[:, mm * P:(mm + 1) * P],
                                lhsT=w_ih[:, kc * G3 + j * P:kc * G3 + (j + 1) * P],
                                rhs=xts[mm][:, kc * P:(kc + 1) * P],
                                start=False,
                                stop=(kc == KC - 1 and mm == ntiles - 1),
                            )
                    iv = apv(ps, 0, [(B, nsteps), (1, B)])
                    ov = apv(dst, j * B, [(JT * B, nsteps), (1, B)])
                    nc.scalar.activation(ov, iv, AF.Copy)

            # ========== R: one GRU layer-step (layout A) ==========
            def r_step(w_hh, bn_row0, ring, prev_slot, gi_ap, tagp):
                cur = 1 - prev_slot
                h_prev = ring[:, prev_slot * P:(prev_slot + 1) * P]
                prz = pp1.tile([P, 8 * B], F32, space="PSUM", tag=f"prz{tagp}")
                pn = pp1.tile([P, 4 * B], F32, space="PSUM", tag=f"pn{tagp}")
                # MM order: r tiles, then n tiles, then z tiles (r/n feed the
                # critical path; z is consumed last)
                for j in range(4):
                    for kc in range(KC):
                        nc.tensor.matmul(
                            prz[:, j * B:(j + 1) * B],
                            lhsT=w_hh[:, kc * G3 + j * P:kc * G3 + (j + 1) * P],
                            rhs=h_prev[:, kc * B:(kc + 1) * B],
                            start=(kc == 0), stop=(kc == KC - 1),
                        )
                for jj in range(4):
                    nc.tensor.matmul(pn[:, jj * B:(jj + 1) * B],
                                     lhsT=bn_all[0:1, (bn_row0 + jj) * P:(bn_row0 + jj + 1) * P],
                                     rhs=ones[:, 0:B], start=True, stop=False)
                    j = 8 + jj
                    for kc in range(KC):
                        nc.tensor.matmul(
                            pn[:, jj * B:(jj + 1) * B],
                            lhsT=w_hh[:, kc * G3 + j * P:kc * G3 + (j + 1) * P],
                            rhs=h_prev[:, kc * B:(kc + 1) * B],
                            start=False, stop=(kc == KC - 1),
                        )
                for j in range(4, 8):
                    for kc in range(KC):
                        nc.tensor.matmul(
                            prz[:, j * B:(j + 1) * B],
                            lhsT=w_hh[:, kc * G3 + j * P:kc * G3 + (j + 1) * P],
                            rhs=h_prev[:, kc * B:(kc + 1) * B],
                            start=(kc == 0), stop=(kc == KC - 1),
                        )
                # r path (critical)
                r_pre = sb.tile([P, 4 * B], BF16, tag=f"rp{tagp}")
                nc.vector.tensor_add(r_pre[:], prz[:, 0:4 * B], gi_ap[:, 0:4 * B])
                rr = sb.tile([P, 4 * B], BF16, tag=f"rr{tagp}")
                nc.scalar.activation(rr[:], r_pre[:], AF.Sigmoid)
                rhn = sb.tile([P, 4 * B], BF16, tag=f"rhn{tagp}")
                nc.vector.tensor_mul(rhn[:], rr[:], pn[:])
                npre = sb.tile([P, 4 * B], BF16, tag=f"npre{tagp}")
                nc.vector.tensor_add(npre[:], rhn[:], gi_ap[:, 8 * B:12 * B])
                cand = sb.tile([P, 4 * B], BF16, tag=f"cand{tagp}")
                nc.scalar.activation(cand[:], npre[:], AF.Tanh)
                # z path (off critical; overlaps tanh)
                z_pre = sb.tile([P, 4 * B], BF16, tag=f"zp{tagp}")
                nc.vector.tensor_add(z_pre[:], prz[:, 4 * B:8 * B],
                                     gi_ap[:, 4 * B:8 * B])
                zz = sb.tile([P, 4 * B], BF16, tag=f"zz{tagp}")
                nc.scalar.activation(zz[:], z_pre[:], AF.Sigmoid)
                zh = sb.tile([P, 4 * B], BF16, tag=f"zh{tagp}")
                nc.gpsimd.tensor_mul(zh[:], zz[:], h_prev[:])
                # blend
                zc = sb.tile([P, 4 * B], BF16, tag=f"zc{tagp}")
                nc.vector.tensor_mul(zc[:], zz[:], cand[:])
                t2 = sb.tile([P, 4 * B], BF16, tag=f"t2{tagp}")
                nc.vector.tensor_sub(t2[:], cand[:], zc[:])
                nc.vector.tensor_add(ring[:, cur * P:(cur + 1) * P], t2[:], zh[:])
                return cur

            # ========== R: gi1 for 1-2 L0 output steps ==========
            def r_gi1(w_ih, bias_rows, src_ring, slots, dst_slot0):
                nsteps = len(slots)
                ntok = nsteps * B
                pg = pp2.tile([P, JT * 2 * B], F32, space="PSUM", tag="pbig")
                for j in range(JT):
                    jb = j * 2 * B
                    nc.tensor.matmul(pg[:, jb:jb + ntok],
                                     lhsT=bias_rows[0:1, j * P:(j + 1) * P],
                                     rhs=ones[:, 0:ntok], start=True, stop=False)
                    for kc in range(KC):
                        if nsteps == 2:
                            nc.tensor.matmul(
                                pg[:, jb:jb + 2 * B],
                                lhsT=w_ih[:, kc * G3 + j * P:kc * G3 + (j + 1) * P],
                                rhs=apv(src_ring, kc * B, [(P, 2), (1, B)]),
                                start=False, stop=(kc == KC - 1),
                            )
                        else:
                            nc.tensor.matmul(
                                pg[:, jb:jb + B],
                                lhsT=w_ih[:, kc * G3 + j * P:kc * G3 + (j + 1) * P],
                                rhs=src_ring[:, slots[0] * P + kc * B:
                                             slots[0] * P + kc * B + B],
                                start=False, stop=(kc == KC - 1),
                            )
                for i in range(nsteps):
                    iv = apv(pg, i * B, [(2 * B, JT), (1, B)])
                    ov = apv(gi1ring, (dst_slot0 + i) * JT * B, [(B, JT), (1, B)])
                    if i == 0:
                        nc.vector.tensor_copy(ov, iv)
                    else:
                        nc.scalar.activation(ov, iv, AF.Copy)

            # ========== proj: one d1 AG chunk on cores 1..7 ==========
            def proj_chunk(k, tok0, ntok):
                dsb = pst.tile([P, 12 * P], BF16, tag="d1sb", bufs=1)
                nc.sync.dma_start(dsb[:], d1_out[k][0:P, :])
                nmt = -(-ntok // P)
                for m2 in range(nmt):
                    mrows = min(P, ntok - m2 * P)
                    msteps = mrows // B
                    for vb in range(NVB):
                        vw = min(VBANK, VC - vb * VBANK)
                        ps = pp3.tile([P, VBANK], F32, space="PSUM", tag="pproj")
                        psv = ps[0:mrows, 0:vw]
                        nc.tensor.matmul(psv, lhsT=ones[:, 0:mrows],
                                         rhs=outb[:, vb * VBANK:vb * VBANK + vw],
                                         start=True, stop=False)
                        for kc in range(KC):
                            lhs = dsb[:, kc * 12 * B + m2 * P:
                                      kc * 12 * B + m2 * P + mrows]
                            nc.tensor.matmul(
                                psv, lhsT=lhs,
                                rhs=outwT[:, kc * VC + vb * VBANK:
                                          kc * VC + vb * VBANK + vw],
                                start=False, stop=(kc == KC - 1),
                            )
                        o = pst.tile([P, VBANK], F32, tag="postage")
                        nc.vector.tensor_copy(o[0:mrows, 0:vw], psv)
                        nc.sync.dma_start(
                            out[tok0 + m2 * P:tok0 + m2 * P + mrows,
                                vb * VBANK:vb * VBANK + vw],
                            o[0:mrows, 0:vw])

            # =========================================================
            # emission
            # =========================================================
            rs = {"s0": 1, "s1": 1}
            gd_sb = [None] * GD_NCHUNK

            def emit_r_steps(t_lo, t_hi, gi0_of, whh0, whh1, wih1, biasL1,
                             bn0_row0, bn1_row0, nsteps_total, d1_sink=None):
                for t in range(t_lo, t_hi):
                    if t < nsteps_total:
                        rs["s0"] = r_step(whh0, bn0_row0, y0ring, rs["s0"],
                                          gi0_of(t), "0")
                        if t % 2 == 1 or t == nsteps_total - 1:
                            g = t // 2
                            slots = ([(t - 1) % 2, t % 2] if t % 2 == 1
                                     else [t % 2])
                            r_gi1(wih1, biasL1, y0ring, slots, (g % 2) * 2)
                    tl1 = t - LAG
                    if 0 <= tl1 < nsteps_total:
                        g = tl1 // 2
                        off = ((g % 2) * 2 + tl1 % 2) * JT * B
                        rs["s1"] = r_step(whh1, bn1_row0, y1ring, rs["s1"],
                                          gi1ring[:, off:off + JT * B], "1")
                        if d1_sink is not None:
                            ck, tloc = d1_sink(tl1)
                            nc.sync.dma_start(
                                apv(d1_in[ck], tloc * B, [(12 * B, KC), (1, B)]),
                                apv(y1ring, rs["s1"] * P, [(B, KC), (1, B)]))

            # ---------------- encoder phase ----------------
            xt_store = {}
            enc_anchors = []

            def d_needs(c):
                tiles = [2 * c, 2 * c + 1]
                if 3 <= c <= 6:
                    kk = c - 3
                    tiles += [N_SRC_TILES + 3 * kk + i for i in range(3)]
                return tiles

            def d_gather_tiles(tiles):
                for m in tiles:
                    emb_ap = d_emb if m < N_SRC_TILES else d_emb2
                    xt_store[m] = d_gather_xT(emb_ap, m)

            cc_pending = []

            def emit_d_chunk(c):
                with tc.If(cid == 1):
                    if c == 0:
                        d_gather_tiles([0, 1])
                    g = gib.tile([P, JT * GDTOK], BF16, tag="gD", name="gD")
                    d_gi0_chunk([xt_store[2 * c], xt_store[2 * c + 1]],
                                w_a, bias1, g)
                    if c + 1 < ENC_NCHUNK:
                        d_gather_tiles([2 * (c + 1), 2 * (c + 1) + 1])
                    nc.sync.dma_start(ge_in[c][:], g[:, 0:JT * ENC_CHUNK * B])
                    if 3 <= c <= 6:
                        kk = c - 3
                        d_gather_tiles([N_SRC_TILES + 3 * kk + i for i in range(3)])
                        xts2 = [xt_store[N_SRC_TILES + 3 * kk + i] for i in range(3)]
                        g2 = gib.tile([P, JT * GDTOK], BF16, tag="gD", name="gD2")
                        d_gi0_chunk(xts2, w_b, bias2, g2)
                        nc.sync.dma_start(gd_in[kk][:], g2[:])
                cc = nc.gpsimd.collective_compute(
                    "AllGather", ALU.bypass, replica_groups=RG2,
                    ins=[ge_in[c].opt()], outs=[ge_out[c].opt()])
                cc_pending.append(cc)
                if c >= 4:
                    cc2 = nc.gpsimd.collective_compute(
                        "AllGather", ALU.bypass, replica_groups=RG2,
                        ins=[gd_in[c - 4].opt()], outs=[gd_out[c - 4].opt()])
                    cc_pending.append(cc2)

            for c2 in range(ENC_NCHUNK // 2):
                emit_d_chunk(2 * c2)
                emit_d_chunk(2 * c2 + 1)
                with tc.If(cid == 0):
                    a = nc.gpsimd.memset(anch[0:1, 0:8], 0.0)
                    enc_anchors.append(a)
                    for ccp in cc_pending:
                        add_dep_helper(a.ins, ccp.ins, sync=False)
                    cc_pending.clear()
                    gsbs = []
                    for c in (2 * c2, 2 * c2 + 1):
                        gsb = gib.tile([P, JT * GDTOK], BF16, tag=f"ge{c % 2}",
                                       name=f"gsb{c}")
                        nc.sync.dma_start(gsb[:, 0:JT * ENC_CHUNK * B],
                                          ge_out[c][P:2 * P, :])
                        gsbs.append(gsb)

                    def gi0_of(t, _g=gsbs, _c2=c2):
                        tloc = t - _c2 * 2 * ENC_CHUNK
                        return _g[tloc // ENC_CHUNK][
                            :, (tloc % ENC_CHUNK) * JT * B:
                            (tloc % ENC_CHUNK + 1) * JT * B]

                    emit_r_steps(c2 * 2 * ENC_CHUNK, (c2 + 1) * 2 * ENC_CHUNK,
                                 gi0_of, w_a, w_b, w_e, bias1, 0, KC, S)

            # ---------------- decoder phase ----------------
            dec_starts = [0]
            for L in DEC_CHUNKS:
                dec_starts.append(dec_starts[-1] + L)

            def d1_sink(t1):
                for ck in range(len(DEC_CHUNKS)):
                    if t1 < dec_starts[ck + 1]:
                        return ck, t1 - dec_starts[ck]
                raise AssertionError

            nK = len(DEC_CHUNKS)
            w_c2 = w_d2 = w_f2 = None
            d1_cc = [None] * nK
            for k in range(nK):
                t_lo, t_hi = dec_starts[k], dec_starts[k + 1]
                with tc.If(cid == 0):
                    if k == 0:
                        # encoder L1 tail
                        emit_r_steps(S, S + LAG, None,
                                     w_a, w_b, w_e, bias1, 0, KC, S)
                        # load decoder weights (own tags, loaded late)
                        w_c2 = wp.tile(WSH, BF16, tag="wc", name="wc2")
                        nc.sync.dma_start(w_c2[:], d_wc[:])
                        w_d2 = wp.tile(WSH, BF16, tag="wd", name="wd2")
                        nc.sync.dma_start(w_d2[:], d_wd[:])
                        w_f2 = wp.tile(WSH, BF16, tag="wf", name="wf2")
                        nc.sync.dma_start(w_f2[:], d_wf[:])
                    a = nc.gpsimd.memset(anch[0:1, 0:8], 0.0)
                    if k >= 2 and d1_cc[k - 2] is not None:
                        add_dep_helper(a.ins, d1_cc[k - 2].ins, sync=False)
                    loads = {0: [0, 1], 1: [2], 2: [3]}.get(k, [])
                    for kk in loads:
                        gg = gib.tile([P, JT * GDTOK], BF16, tag=f"ge{kk % 2}",
                                      name=f"gdl{kk}")
                        nc.sync.dma_start(gg[:], gd_out[kk][P:2 * P, :])
                        gd_sb[kk % 2] = (kk, gg)

                    def gi0d_of(t):
                        kk = t // 12
                        tloc = t - kk * 12
                        kk2, gg = gd_sb[kk % 2]
                        assert kk2 == kk
                        return gg[:, tloc * JT * B:(tloc + 1) * JT * B]

                    emit_r_steps(t_lo, t_hi, gi0d_of,
                                 w_c2, w_d2, w_f2, bias2, 2 * KC, 3 * KC, TD,
                                 d1_sink=d1_sink)
                    if k == nK - 1:
                        emit_r_steps(TD, TD + LAG, None,
                                     w_c2, w_d2, w_f2, bias2, 2 * KC, 3 * KC, TD,
                                     d1_sink=d1_sink)
                if k >= 1:
                    kk = k - 1
                    d1_cc[kk] = nc.gpsimd.collective_compute(
                        "AllGather", ALU.bypass, replica_groups=RG,
                        ins=[d1_in[kk].opt()], outs=[d1_out[kk].opt()])
                    with tc.If(cid == 0) as cp:
                        pass
                    with cp.Else():
                        proj_chunk(kk, dec_starts[kk] * B,
                                   DEC_CHUNKS[kk] * B)
            # final d1 chunk
            kk = nK - 1
            nc.gpsimd.collective_compute(
                "AllGather", ALU.bypass, replica_groups=RG,
                ins=[d1_in[kk].opt()], outs=[d1_out[kk].opt()])
            with tc.If(cid == 0) as cp:
                pass
            with cp.Else():
                proj_chunk(kk, dec_starts[kk] * B, DEC_CHUNKS[kk] * B)

    nc.compile()
    _CACHE["nc"] = nc
    return nc


# ================================================================ host side
def _bf16(x):
    return np.asarray(x, dtype=np.float32).astype(ml_dtypes.bfloat16)


def pack_wT(W):
    """W [out_dim, in_dim] -> [128, (in_dim/128)*out_dim] bf16 stationary pack."""
    W = np.asarray(W, np.float32)
    out_dim, in_dim = W.shape
    kc = in_dim // P
    Wt = W.T.reshape(kc, P, out_dim)
    return _bf16(np.concatenate([Wt[i] for i in range(kc)], axis=1))


def pack_bias_rows(bih, bhh):
    """gi-side bias rows [12, 128]: rz tiles bih+bhh, n tiles bih only."""
    bih = np.asarray(bih, np.float32)
    bhh = np.asarray(bhh, np.float32)
    rows = np.zeros((JT, P), np.float32)
    for j in range(JT):
        seg = bih[j * P:(j + 1) * P].copy()
        if j < 8:
            seg = seg + bhh[j * P:(j + 1) * P]
        rows[j] = seg
    return _bf16(rows.reshape(1, JT * P))


def pack_bn_rows(bhh):
    return _bf16(np.asarray(bhh, np.float32)[2 * H:3 * H].reshape(1, KC * P))


def pack_idx(src, trg):
    src_f = np.asarray(src, np.int64).reshape(-1)
    trg_f = np.asarray(trg, np.int64)[:TD].reshape(-1)
    trg_f = np.concatenate([trg_f, np.zeros(N_TRG_TILES * P - TD * B, np.int64)])
    cols = [src_f[m * P:(m + 1) * P] for m in range(N_SRC_TILES)]
    cols += [trg_f[m * P:(m + 1) * P] for m in range(N_TRG_TILES)]
    return np.stack(cols, axis=1).astype(np.int32)


def pack_outwT(out_W, core):
    vbase = (core - 1) * VC
    sl = np.zeros((VC, H), np.float32)
    real = np.asarray(out_W, np.float32)[vbase:min(vbase + VC, V)]
    sl[:real.shape[0]] = real
    slT = sl.T.reshape(KC, P, VC)
    return _bf16(np.concatenate([slT[i] for i in range(KC)], axis=1))


def pack_outb(out_b, core):
    vbase = (core - 1) * VC
    sl = np.zeros((VC,), np.float32)
    real = np.asarray(out_b, np.float32)[vbase:min(vbase + VC, V)]
    sl[:real.shape[0]] = real
    return _bf16(sl[None, :])


def make_in_maps(inputs):
    z_w = np.zeros((P, KC * G3), ml_dtypes.bfloat16)
    z_emb = np.zeros((V, E), ml_dtypes.bfloat16)
    z_owt = np.zeros((P, KC * VC), ml_dtypes.bfloat16)
    z_ob = np.zeros((1, VC), ml_dtypes.bfloat16)
    z_bn = np.zeros((1, 4 * KC * P), ml_dtypes.bfloat16)
    z_b = np.zeros((1, JT * P), ml_dtypes.bfloat16)
    z_idx = np.zeros((P, N_SRC_TILES + N_TRG_TILES), np.int32)
    ident = _bf16(np.eye(P))
    ones_arr = np.ones((1, GDTOK), ml_dtypes.bfloat16)

    emb_e = _bf16(inputs["enc_emb"])
    emb_d = _bf16(inputs["dec_emb"])
    idx = pack_idx(inputs["src"], inputs["trg"])

    in_maps = []
    for core in range(N_CORES):
        m = {
            "wa": z_w, "wb": z_w, "wc": z_w, "wd": z_w, "we": z_w, "wf": z_w,
            "bn_all": z_bn, "bias1": z_b, "bias2": z_b,
            "emb": z_emb, "emb2": z_emb, "idx": z_idx,
            "outwT": z_owt, "outb": z_ob,
            "ident": ident, "ones": ones_arr,
        }
        if core == 0:
            m["wa"] = pack_wT(inputs["enc_Whh0"])
            m["wb"] = pack_wT(inputs["enc_Whh1"])
            m["wc"] = pack_wT(inputs["dec_Whh0"])
            m["wd"] = pack_wT(inputs["dec_Whh1"])
            m["we"] = pack_wT(inputs["enc_Wih1"])
            m["wf"] = pack_wT(inputs["dec_Wih1"])
            m["bn_all"] = np.concatenate([
                pack_bn_rows(inputs["enc_bhh0"]),
                pack_bn_rows(inputs["enc_bhh1"]),
                pack_bn_rows(inputs["dec_bhh0"]),
                pack_bn_rows(inputs["dec_bhh1"]),
            ], axis=1)
            m["bias1"] = pack_bias_rows(inputs["enc_bih1"], inputs["enc_bhh1"])
            m["bias2"] = pack_bias_rows(inputs["dec_bih1"], inputs["dec_bhh1"])
        if core == 1:
            m["wa"] = pack_wT(inputs["enc_Wih0"])
            m["wb"] = pack_wT(inputs["dec_Wih0"])
            m["bias1"] = pack_bias_rows(inputs["enc_bih0"], inputs["enc_bhh0"])
            m["bias2"] = pack_bias_rows(inputs["dec_bih0"], inputs["dec_bhh0"])
            m["emb"] = emb_e
            m["emb2"] = emb_d
            m["idx"] = idx
        if core >= 1:
            m["outwT"] = pack_outwT(inputs["out_W"], core)
            m["outb"] = pack_outb(inputs["out_b"], core)
        in_maps.append(m)
    return in_maps


def kernel(**inputs):
    global LAST_RESULTS
    nc = build_program()
    in_maps = make_in_maps(inputs)
    kw = {}
    if TRACE:
        try:
            import ntff_shim
            ntff_shim.install()
        except Exception:
            pass
        kw["trace"] = True
        if TRACE_TMPDIR:
            kw["tmpdir"] = TRACE_TMPDIR
    res = bass_utils.run_bass_kernel_spmd(nc, in_maps,
                                          core_ids=list(range(N_CORES)), **kw)
    LAST_RESULTS = res
    parts = [np.asarray(res.results[c]["out"]) for c in range(1, N_CORES)]
    logits = np.concatenate(parts, axis=1)[:, :V]
    return logits.reshape(TD, B, V).astype(np.float32)
